# revision 14
# baseline (speedup 1.0000x reference)
"""ConvSP (SPH message-passing conv) Trainium2 kernel.

Problem (per full input): B=2 batches, N=8192 particles, M=8192 queries,
K=32 neighbors, C=16 in channels, O=16 out channels, 27 kernel cells.

    out[b,m,o] = bias[o] + sum_{e,k,c} kv(b,m,e,k) * data[b, nbr[b,m,k], c] * W[o,c,e]
    kv = relu(1 - sqrt(|qloc + off_e - loc_nbr|^2 + 1e-12)/R)^3

Sharding: 8 cores = 2 batches x 4 query-quarters (2048 queries each), SPMD.

Per-core dataflow (chunk = 4 queries m=4j+t; partition (t,k) = t*32+k):
  - records rec64[n] = [lx,ly,lz, data0..15, pad..] (256B rows) built by
    DRAM->DRAM DMAs; neighbor records fetched with dma_gather: the natural
    flat neighbor stream s = m*32+k lands at out[(t,k), j, :]
    (p = s%128, chunk = s//128).
  - distances on DVE via the separable cell-offset factorization
    d2 = |delta|^2 + sum_axis(2*off*delta + off^2); kv = relu(1-d/R)^3
    via ACT sqrt + ACT relu-affine + DVE squares.
  - data c-fields copied into a block-diagonal "slotted" tile (zeros
    elsewhere, memset once); acc[(t,c), e] per chunk via one PE matmul
    (lhsT = slotted data [128, 64], rhs = dense kv [128, 27]).
  - out[(t,o), j] via 27 accumulated PE matmuls against host-prepped
    block-diagonal W; bias fused in the ACT PSUM->SBUF copy; PE transpose
    to [j, (t,o)] for a contiguous store.
"""
import os
import sys
import numpy as np
from contextlib import ExitStack

sys.path.insert(0, "/opt/trn_rl_repo")

import concourse.bass as bass
import concourse.bacc as bacc
import concourse.mybir as mybir
import concourse.tile as tile
from concourse.masks import make_identity
from concourse.bass_utils import run_bass_kernel_spmd

F32 = mybir.dt.float32
F32R = mybir.dt.float32r
BF16 = mybir.dt.bfloat16
I32 = mybir.dt.int32
I16 = mybir.dt.int16
AF = mybir.ActivationFunctionType

P = 128          # partitions
NQ = 2048        # queries per core
N = 8192         # particles per batch
K = 32           # neighbors
C = 16           # in channels
O = 16           # out channels
D = 3
E = 27           # cells
REC = 64         # padded record fields (f32): lx,ly,lz,d0..15,pad -> 256B
T = 4            # queries per chunk
J = NQ // T      # chunks per core = 512
JS = 64          # chunks per gather block == compute subtile
NSUB = J // JS   # 8
RADIUS = 0.1
DIL = 0.05
STAGE = int(os.environ.get("CONVSP_STAGE", "5"))


def build_program():
    nc = bacc.Bacc("TRN2", target_bir_lowering=False, debug=False,
                   num_devices=8)

    qlocs_d = nc.declare_dram_parameter("qlocs", [NQ, D], F32, isOutput=False)
    nbrs_d = nc.declare_dram_parameter("nbrs", [NQ, K], I32, isOutput=False)
    locs_d = nc.declare_dram_parameter("locs", [N, D], F32, isOutput=False)
    data_d = nc.declare_dram_parameter("data", [N, C], F32, isOutput=False)
    wbd_d = nc.declare_dram_parameter("wbd", [64, E * 64], BF16, isOutput=False)
    bias4_d = nc.declare_dram_parameter("bias4", [64], F32, isOutput=False)
    out_d = nc.declare_dram_parameter("out", [NQ, O], F32, isOutput=True)

    rec_d = nc.dram_tensor("rec_scratch", [N, REC], F32)
    q4_d = nc.dram_tensor("q4_scratch", [T, J, D], F32)
    w16_d = nc.dram_tensor("w16_scratch", [16, J * 8], I16)

    with tile.TileContext(nc) as tc:
        with ExitStack() as ctx:
            _build(ctx, tc, qlocs_d, nbrs_d, locs_d, data_d, wbd_d, bias4_d,
                   out_d, rec_d, q4_d, w16_d)
    nc.finalize()
    return nc


def _build(ctx, tc, qlocs_d, nbrs_d, locs_d, data_d, wbd_d, bias4_d,
           out_d, rec_d, q4_d, w16_d):
    nc = tc.nc

    consts = ctx.enter_context(tc.tile_pool(name="consts", bufs=1))
    prep = ctx.enter_context(tc.tile_pool(name="prep", bufs=1))
    gpool = ctx.enter_context(tc.tile_pool(name="gpool", bufs=1))
    dve = ctx.enter_context(tc.tile_pool(name="dve", bufs=2))
    kvp = ctx.enter_context(tc.tile_pool(name="kv", bufs=2))
    accs = ctx.enter_context(tc.tile_pool(name="accs", bufs=1))
    outs = ctx.enter_context(tc.tile_pool(name="outs", bufs=2))
    accps = ctx.enter_context(tc.tile_pool(name="accps", bufs=4, space="PSUM"))
    outps = ctx.enter_context(tc.tile_pool(name="outps", bufs=2, space="PSUM"))
    trps = ctx.enter_context(tc.tile_pool(name="trps", bufs=2, space="PSUM"))

    # ---------------- constants ----------------
    oxc = consts.tile([P, D], F32)      # 2*off(e)
    ox2c = consts.tile([P, D], F32)     # off(e)^2
    for i in range(D):
        off = (i - 1) * DIL
        nc.vector.memset(oxc[:, i:i + 1], 2.0 * off)
        nc.vector.memset(ox2c[:, i:i + 1], off * off)
    epsb = consts.tile([P, 1], F32)
    nc.vector.memset(epsb[:], 1e-12)
    oneb = consts.tile([P, 1], F32)
    nc.vector.memset(oneb[:], 1.0)
    ident = consts.tile([64, 64], F32)
    make_identity(nc, ident[:])
    bias4 = consts.tile([64, 1], F32)
    nc.sync.dma_start(bias4[:], bias4_d[:].rearrange("(p o) -> p o", o=1))
    wbd = consts.tile([64, E * 64], BF16)
    nc.scalar.dma_start(wbd[:], wbd_d[:])

    # ---------------- stage A ----------------
    # rec64[n] = [lx,ly,lz, d0..15, garbage...] built in SBUF, one big write
    tl_locs = prep.tile([P, (N // P) * D], F32)
    nc.scalar.dma_start(tl_locs[:], locs_d[:].rearrange("(p a) d -> p (a d)", p=P))
    tl_data = prep.tile([P, (N // P) * C], F32)
    nc.scalar.dma_start(tl_data[:], data_d[:].rearrange("(p a) c -> p (a c)", p=P))
    tl_rec = prep.tile([P, (N // P) * REC], F32)
    rec_v = tl_rec[:].rearrange("p (a f) -> p a f", f=REC)
    locs_v = tl_locs[:].rearrange("p (a d) -> p a d", d=D)
    data_v = tl_data[:].rearrange("p (a c) -> p a c", c=C)
    for d in range(D):
        nc.vector.tensor_copy(rec_v[:, :, d], locs_v[:, :, d])
    nc.vector.tensor_copy(rec_v[:, :, D:D + C], data_v[:, :, :])
    nc.vector.memset(rec_v[:, :, D + C:REC], 0.0)
    nc.scalar.dma_start(rec_d[:].rearrange("(p a) f -> p (a f)", p=P), tl_rec[:])

    # q4_d[t, j, d] = qlocs[4j+t, d] via DRAM->DRAM, then broadcast to qT3
    nc.sync.dma_start(q4_d[:], qlocs_d[:].rearrange("(j t) d -> t j d", t=T))
    qT3 = consts.tile([P, J * D], F32)
    for t in range(T):
        src = q4_d[t].rearrange("j d -> (j d)")
        eng = nc.sync if t % 2 == 0 else nc.scalar
        eng.dma_start(qT3[t * K:(t + 1) * K, :], src.partition_broadcast(K))

    # wrap-format gather indices W16[r, 2m+khi] = nbrs[m, khi*16+r], int16,
    # replicated into all 8 gpsimd core groups.
    n1 = prep.tile([P, J], I32)
    nc.sync.dma_start(n1[:].rearrange("p (a k) -> p a k", k=K),
                      nbrs_d[:].rearrange("(a p) k -> p a k", p=P))
    t1 = prep.tile([P, J], I32)
    nc.vector.transpose(t1[:], n1[:])   # t1[(pb,k),(a,j)] = nbrs[a*128+pb*32+j, k]
    # engine APs need 32-aligned partition bases: shift the khi=1 half-rows
    # (k=16..32 of each pb block) down to base pb*32 via SBUF->SBUF DMA.
    t1s = prep.tile([P, J], I32)
    for pb in range(4):
        nc.sync.dma_start(t1s[pb * K:pb * K + 16, :],
                          t1[pb * K + 16:(pb + 1) * K, :])
    # W16[r, col] = stream[col*16+r]; col = 2m+khi, m = a*128 + pb*32 + j
    w16s = prep.tile([P, J * 8], I16)
    w16sv = w16s[:].rearrange("p (a pj two) -> p a pj two", a=16, pj=P, two=2)
    t1v = t1[:].rearrange("p (a j) -> p a j", j=K)
    t1sv = t1s[:].rearrange("p (a j) -> p a j", j=K)
    for pb in range(4):
        nc.vector.tensor_copy(w16sv[0:16, :, pb * K:(pb + 1) * K, 0],
                              t1v[pb * K:pb * K + 16, :, :])
        nc.vector.tensor_copy(w16sv[0:16, :, pb * K:(pb + 1) * K, 1],
                              t1sv[pb * K:pb * K + 16, :, :])
    # replicate the 16-row wrap into all 8 gpsimd core groups via DRAM bounce
    nc.sync.dma_start(w16_d[:], w16s[0:16, :])
    w16 = consts.tile([P, J * 8], I16)  # [128, 4096]
    nc.sync.dma_start(w16[:], w16_d[:].partition_broadcast(8))

    # ---------------- gather + slotted buffers ----------------
    gbufs = [gpool.tile([P, JS * REC], F32, tag=f"g{i}", name=f"g{i}")
             for i in range(2)]
    sbufs = [gpool.tile([P, JS * 64], BF16, tag=f"s{i}", name=f"s{i}")
             for i in range(2)]
    nc.vector.memset(sbufs[0][:], 0.0)
    nc.vector.memset(sbufs[1][:], 0.0)

    def gather_block(s):
        if STAGE < 2:
            return
        g = gbufs[s % 2]
        nc.gpsimd.dma_gather(
            out_ap=g[:].rearrange("p (j f) -> p j f", f=REC),
            in_ap=rec_d[:],
            idxs_ap=w16[:, s * (JS * 8):(s + 1) * (JS * 8)],
            num_idxs=JS * P,
            num_idxs_reg=JS * P,
            elem_size=REC,
            single_packet=False,
        )

    def subtile(s):
        if STAGE < 2:
            return
        g = gbufs[s % 2]
        sbd = sbufs[s % 2]
        gv = g[:].rearrange("p (j f) -> p j f", f=REC)
        if STAGE < 3:
            if s == 0:
                nc.sync.dma_start(out_d[0:64, :],
                                  gv[0:64, 0, 0:O])
            return

        # --- block-diag data: copy c-fields of t-block rows into slot t
        sv = sbd[:].rearrange("p (j s c) -> p j s c", s=T, c=C)
        for t in range(T):
            src = gv[t * K:(t + 1) * K, :, D:D + C]
            dst = sv[t * K:(t + 1) * K, :, t, :]
            if t < 2:
                nc.vector.tensor_copy(dst, src)
            else:
                nc.scalar.activation(dst, src, AF.Copy)

        # --- delta = q - l
        d3 = dve.tile([P, JS * D], F32, tag="d3")
        q_v = qT3[:].rearrange("p (j d) -> p j d", d=D)
        nc.vector.tensor_sub(
            d3[:].rearrange("p (j d) -> p j d", d=D),
            q_v[:, s * JS:(s + 1) * JS, :],
            gv[:, :, 0:D])

        d3v = d3[:].rearrange("p (j d) -> p j d", d=D)
        dx, dy, dz = d3v[:, :, 0], d3v[:, :, 1], d3v[:, :, 2]

        # --- s2 = dx^2+dy^2+dz^2
        s2 = dve.tile([P, JS], F32, tag="s2")
        tmp = dve.tile([P, JS], F32, tag="tmp")
        nc.vector.tensor_mul(s2[:], dx, dx)
        nc.vector.tensor_mul(tmp[:], dy, dy)
        nc.vector.tensor_add(s2[:], s2[:], tmp[:])
        nc.vector.tensor_mul(tmp[:], dz, dz)
        nc.vector.tensor_add(s2[:], s2[:], tmp[:])

        # --- per-axis terms p*[j,e] = 2*off*d + off^2 (+ s2 on x)
        def axis_term(dcomp, add_s2, tg):
            pt = dve.tile([P, JS * D], F32, tag=tg)
            ptv = pt[:].rearrange("p (j e) -> p j e", e=D)
            din = dcomp.unsqueeze(2).broadcast_to((P, JS, D))
            oc = oxc[:].unsqueeze(1).broadcast_to((P, JS, D))
            o2 = ox2c[:].unsqueeze(1).broadcast_to((P, JS, D))
            nc.vector.tensor_mul(ptv, din, oc)
            nc.vector.tensor_add(ptv, ptv, o2)
            if add_s2:
                s2b = s2[:].unsqueeze(2).broadcast_to((P, JS, D))
                nc.vector.tensor_add(ptv, ptv, s2b)
            return pt

        pxe = axis_term(dx, True, "pxe")
        pye = axis_term(dy, False, "pye")
        pze = axis_term(dz, False, "pze")

        # --- u2[j,ex,ey] = pxe+pye ; d2[j,ex,ey,ez] = u2+pze
        u2 = dve.tile([P, JS * 9], F32, tag="u2")
        u2v = u2[:].rearrange("p (j a b) -> p j a b", a=D, b=D)
        nc.vector.tensor_add(
            u2v,
            pxe[:].rearrange("p (j a) -> p j a", a=D).unsqueeze(3)
                  .broadcast_to((P, JS, D, D)),
            pye[:].rearrange("p (j b) -> p j b", b=D).unsqueeze(2)
                  .broadcast_to((P, JS, D, D)))
        d2 = kvp.tile([P, JS * E], F32, tag="d2")
        d2v = d2[:].rearrange("p (j a b) -> p j a b", a=9, b=D)
        nc.vector.tensor_add(
            d2v,
            u2[:].rearrange("p (j a) -> p j a", a=9).unsqueeze(3)
                 .broadcast_to((P, JS, 9, D)),
            pze[:].rearrange("p (j b) -> p j b", b=D).unsqueeze(2)
                  .broadcast_to((P, JS, 9, D)))

        # --- kv = relu(1 - sqrt(d2+eps)/R)^3
        nc.scalar.activation(d2[:], d2[:], AF.Sqrt, bias=epsb[:])
        nc.scalar.activation(d2[:], d2[:], AF.Relu, bias=oneb[:],
                             scale=-1.0 / RADIUS)
        sq = kvp.tile([P, JS * E], F32, tag="sq")
        nc.vector.tensor_mul(sq[:], d2[:], d2[:])
        kv = kvp.tile([P, JS * E], BF16, tag="kvt")
        nc.vector.tensor_mul(kv[:], sq[:], d2[:])

        if STAGE < 4:
            if s == 0:
                nc.sync.dma_start(out_d[0:64, :],
                                  kv[0:64, 0:O])
            return

        # --- acc[(t,c), e] per chunk on PE (bf16: single-pass PE matmul)
        kvv = kv[:].rearrange("p (j e) -> p j e", e=E)
        acc_sb = acc4[s % 4]
        for grp in range(JS // 16):
            ap_ps = accps.tile([64, 16 * E], F32, tag="accps")
            for jl in range(16):
                jj = grp * 16 + jl
                nc.tensor.matmul(ap_ps[:, jl * E:(jl + 1) * E],
                                 sbd[:, jj * 64:(jj + 1) * 64],
                                 kvv[:, jj, :],
                                 start=True, stop=True)
            nc.scalar.activation(acc_sb[:, grp * 16 * E:(grp + 1) * 16 * E],
                                 ap_ps[:], AF.Copy)

        if STAGE < 5:
            if s == 0:
                nc.sync.dma_start(out_d[0:64, :], acc_sb[:, 0:O])
            return

    def final_group(fg):
        # --- out[(t,o), jtot] = sum_e Wbd_e @ acc_e over 4 subtiles (256 cols)
        JT = 4 * JS
        op = outps.tile([64, JT], F32, tag="outps")
        accv = accbig[:].rearrange("p (jt e) -> p jt e", e=E)
        for e in range(E):
            nc.tensor.matmul(op[:], wbd[:, e * 64:(e + 1) * 64],
                             accv[:, :, e],
                             start=(e == 0), stop=(e == E - 1))
        osb = outs.tile([64, JT], F32, tag="osb")
        nc.scalar.activation(osb[:], op[:], AF.Identity, bias=bias4[:])

        # --- transpose to [j, (t,o)] and store contiguously
        out_v = out_d[:].rearrange("(s j t) o -> s j (t o)", s=NSUB, t=T)
        for q in range(4):
            trp = trps.tile([64, 64], F32, tag="trp")
            nc.tensor.transpose(trp[:], osb[:, q * 64:(q + 1) * 64], ident[:])
            trs = outs.tile([64, 64], F32, tag="trs")
            nc.scalar.activation(trs[:], trp[:], AF.Copy)
            nc.sync.dma_start(out_v[fg * 4 + q], trs[:])

    if STAGE < 2:
        nc.sync.dma_start(out_d[0:128, 0:8],
                          w16[:, 0:8].bitcast(F32).rearrange("p (a b) -> p a b", b=1)[:, :, 0]
                          if False else qT3[:, 0:8])
    accbig = accs.tile([64, 4 * JS * E], BF16, tag="accbig", name="accbig")
    acc4 = [accbig[:, i * JS * E:(i + 1) * JS * E] for i in range(4)]

    # ---------------- pipeline ----------------
    gather_block(0)
    for s in range(NSUB):
        if s + 1 < NSUB:
            gather_block(s + 1)
        subtile(s)
        if STAGE >= 5 and s % 4 == 3:
            final_group(s // 4)


_PROGRAM = None


def _get_program():
    global _PROGRAM
    if _PROGRAM is None:
        _PROGRAM = build_program()
    return _PROGRAM


def kernel(qlocs, locs, data, neighbors, weight, bias):
    B, M = qlocs.shape[0], qlocs.shape[1]
    assert (B, M) == (2, 8192)
    ncores = 8

    # host-side constant/layout prep: block-diagonal weights + replicated bias
    wbd = np.zeros((E, 64, 64), np.float32)
    w = np.asarray(weight, np.float32)           # [O, C, E]
    for t in range(T):
        # wbd[e, (t,c), (t,o)] = w[o, c, e]
        wbd[:, t * C:(t + 1) * C, t * O:(t + 1) * O] = w.transpose(2, 1, 0)
    import ml_dtypes
    wbd = np.ascontiguousarray(
        wbd.transpose(1, 0, 2).reshape(64, E * 64)).astype(ml_dtypes.bfloat16)
    bias4 = np.tile(np.asarray(bias, np.float32), T)

    in_maps = []
    for cid in range(ncores):
        b, qq = cid // 4, cid % 4
        sl = slice(qq * NQ, (qq + 1) * NQ)
        in_maps.append({
            "qlocs": np.ascontiguousarray(qlocs[b, sl], np.float32),
            "nbrs": np.ascontiguousarray(neighbors[b, sl], np.int32),
            "locs": np.ascontiguousarray(locs[b], np.float32),
            "data": np.ascontiguousarray(data[b], np.float32),
            "wbd": wbd,
            "bias4": bias4,
        })

    nc = _get_program()
    res = run_bass_kernel_spmd(nc, in_maps, list(range(ncores)),
                               trace=bool(int(os.environ.get("CONVSP_TRACE", "0"))))
    out = np.zeros((B, M, O), np.float32)
    for cid in range(ncores):
        b, qq = cid // 4, cid % 4
        out[b, qq * NQ:(qq + 1) * NQ] = res.results[cid]["out"]
    kernel.last_results = res
    return out



# revision 19
# speedup vs baseline: 1.0045x; 1.0045x over previous
"""ConvSP (SPH message-passing conv) Trainium2 kernel.

Problem (per full input): B=2 batches, N=8192 particles, M=8192 queries,
K=32 neighbors, C=16 in channels, O=16 out channels, 27 kernel cells.

    out[b,m,o] = bias[o] + sum_{e,k,c} kv(b,m,e,k) * data[b, nbr[b,m,k], c] * W[o,c,e]
    kv = relu(1 - sqrt(|qloc + off_e - loc_nbr|^2 + 1e-12)/R)^3

Sharding: 8 cores = 2 batches x 4 query-quarters (2048 queries each), SPMD.

Per-core dataflow (chunk = 4 queries m=4j+t; partition (t,k) = t*32+k):
  - records rec64[n] = [lx,ly,lz, data0..15, pad..] (256B rows) built by
    DRAM->DRAM DMAs; neighbor records fetched with dma_gather: the natural
    flat neighbor stream s = m*32+k lands at out[(t,k), j, :]
    (p = s%128, chunk = s//128).
  - distances on DVE via the separable cell-offset factorization
    d2 = |delta|^2 + sum_axis(2*off*delta + off^2); kv = relu(1-d/R)^3
    via ACT sqrt + ACT relu-affine + DVE squares.
  - data c-fields copied into a block-diagonal "slotted" tile (zeros
    elsewhere, memset once); acc[(t,c), e] per chunk via one PE matmul
    (lhsT = slotted data [128, 64], rhs = dense kv [128, 27]).
  - out[(t,o), j] via 27 accumulated PE matmuls against host-prepped
    block-diagonal W; bias fused in the ACT PSUM->SBUF copy; PE transpose
    to [j, (t,o)] for a contiguous store.
"""
import os
import sys
import numpy as np
from contextlib import ExitStack

sys.path.insert(0, "/opt/trn_rl_repo")

import concourse.bass as bass
import concourse.bacc as bacc
import concourse.mybir as mybir
import concourse.tile as tile
from concourse.masks import make_identity
from concourse.bass_utils import run_bass_kernel_spmd

F32 = mybir.dt.float32
F32R = mybir.dt.float32r
BF16 = mybir.dt.bfloat16
I32 = mybir.dt.int32
I16 = mybir.dt.int16
AF = mybir.ActivationFunctionType

P = 128          # partitions
NQ = 2048        # queries per core
N = 8192         # particles per batch
K = 32           # neighbors
C = 16           # in channels
O = 16           # out channels
D = 3
E = 27           # cells
REC = 64         # padded record fields (f32): lx,ly,lz,d0..15,pad -> 256B
T = 4            # queries per chunk
J = NQ // T      # chunks per core = 512
JS = 64          # chunks per gather block == compute subtile
NSUB = J // JS   # 8
RADIUS = 0.1
DIL = 0.05
STAGE = int(os.environ.get("CONVSP_STAGE", "5"))


def build_program():
    nc = bacc.Bacc("TRN2", target_bir_lowering=False, debug=False,
                   num_devices=8)

    qlocs_d = nc.declare_dram_parameter("qlocs", [NQ, D], F32, isOutput=False)
    nbrs_d = nc.declare_dram_parameter("nbrs", [NQ, K], I32, isOutput=False)
    locs_d = nc.declare_dram_parameter("locs", [N, D], F32, isOutput=False)
    data_d = nc.declare_dram_parameter("data", [N, C], F32, isOutput=False)
    wbd_d = nc.declare_dram_parameter("wbd", [64, E * 64], BF16, isOutput=False)
    bias4_d = nc.declare_dram_parameter("bias4", [64], F32, isOutput=False)
    out_d = nc.declare_dram_parameter("out", [NQ, O], F32, isOutput=True)

    rec_d = nc.dram_tensor("rec_scratch", [N, REC], F32)
    q4_d = nc.dram_tensor("q4_scratch", [T, J, D], F32)
    w16_d = nc.dram_tensor("w16_scratch", [16, J * 8], I16)

    with tile.TileContext(nc) as tc:
        with ExitStack() as ctx:
            _build(ctx, tc, qlocs_d, nbrs_d, locs_d, data_d, wbd_d, bias4_d,
                   out_d, rec_d, q4_d, w16_d)
    nc.finalize()
    return nc


def _build(ctx, tc, qlocs_d, nbrs_d, locs_d, data_d, wbd_d, bias4_d,
           out_d, rec_d, q4_d, w16_d):
    nc = tc.nc

    consts = ctx.enter_context(tc.tile_pool(name="consts", bufs=1))
    prep = ctx.enter_context(tc.tile_pool(name="prep", bufs=1))
    gpool = ctx.enter_context(tc.tile_pool(name="gpool", bufs=1))
    dve = ctx.enter_context(tc.tile_pool(name="dve", bufs=2))
    kvp = ctx.enter_context(tc.tile_pool(name="kv", bufs=2))
    accs = ctx.enter_context(tc.tile_pool(name="accs", bufs=1))
    outs = ctx.enter_context(tc.tile_pool(name="outs", bufs=2))
    accps = ctx.enter_context(tc.tile_pool(name="accps", bufs=4, space="PSUM"))
    outps = ctx.enter_context(tc.tile_pool(name="outps", bufs=2, space="PSUM"))
    trps = ctx.enter_context(tc.tile_pool(name="trps", bufs=2, space="PSUM"))

    # ---------------- constants ----------------
    oxc = consts.tile([P, D], F32)      # 2*off(e)
    ox2c = consts.tile([P, D], F32)     # off(e)^2
    for i in range(D):
        off = (i - 1) * DIL
        nc.vector.memset(oxc[:, i:i + 1], 2.0 * off)
        nc.vector.memset(ox2c[:, i:i + 1], off * off)
    epsb = consts.tile([P, 1], F32)
    nc.vector.memset(epsb[:], 1e-12)
    oneb = consts.tile([P, 1], F32)
    nc.vector.memset(oneb[:], 1.0)
    ident = consts.tile([64, 64], F32)
    make_identity(nc, ident[:])
    bias4 = consts.tile([64, 1], F32)
    nc.sync.dma_start(bias4[:], bias4_d[:].rearrange("(p o) -> p o", o=1))
    wbd = consts.tile([64, E * 64], BF16)
    nc.scalar.dma_start(wbd[:], wbd_d[:])

    # ---------------- stage A ----------------
    # rec64[n] = [lx,ly,lz, d0..15, garbage...] built in SBUF, one big write
    tl_locs = prep.tile([P, (N // P) * D], F32)
    nc.scalar.dma_start(tl_locs[:], locs_d[:].rearrange("(p a) d -> p (a d)", p=P))
    tl_data = prep.tile([P, (N // P) * C], F32)
    nc.scalar.dma_start(tl_data[:], data_d[:].rearrange("(p a) c -> p (a c)", p=P))
    tl_rec = prep.tile([P, (N // P) * REC], F32)
    rec_v = tl_rec[:].rearrange("p (a f) -> p a f", f=REC)
    locs_v = tl_locs[:].rearrange("p (a d) -> p a d", d=D)
    data_v = tl_data[:].rearrange("p (a c) -> p a c", c=C)
    for d in range(D):
        nc.vector.tensor_copy(rec_v[:, :, d], locs_v[:, :, d])
    # data fields stored bf16 at byte offset 12 so slot copies are raw
    # byte moves (DMA-able); trailing record bytes stay garbage (never read)
    rec_b = tl_rec[:].bitcast(BF16).rearrange("p (a f) -> p a f", f=REC * 2)
    nc.vector.tensor_copy(rec_b[:, :, 2 * D:2 * D + C], data_v[:, :, :])
    rec_st = rec_d[:].rearrange("(p a) f -> p (a f)", p=P)
    half = (N // P) * REC // 2
    nc.scalar.dma_start(rec_st[:, 0:half], tl_rec[:, 0:half])
    nc.sync.dma_start(rec_st[:, half:], tl_rec[:, half:])

    # q4_d[t, j, d] = qlocs[4j+t, d] via DRAM->DRAM, then broadcast to qT3
    nc.sync.dma_start(q4_d[:], qlocs_d[:].rearrange("(j t) d -> t j d", t=T))
    qT3 = consts.tile([P, J * D], F32)
    for t in range(T):
        src = q4_d[t].rearrange("j d -> (j d)")
        eng = nc.sync if t % 2 == 0 else nc.scalar
        eng.dma_start(qT3[t * K:(t + 1) * K, :], src.partition_broadcast(K))

    # wrap-format gather indices W16[r, 2m+khi] = nbrs[m, khi*16+r], int16,
    # replicated into all 8 gpsimd core groups.
    n1 = prep.tile([P, J], I32)
    nc.sync.dma_start(n1[:].rearrange("p (a k) -> p a k", k=K),
                      nbrs_d[:].rearrange("(a p) k -> p a k", p=P))
    t1 = prep.tile([P, J], I32)
    nc.vector.transpose(t1[:], n1[:])   # t1[(pb,k),(a,j)] = nbrs[a*128+pb*32+j, k]
    # engine APs need 32-aligned partition bases: shift the khi=1 half-rows
    # (k=16..32 of each pb block) down to base pb*32 via SBUF->SBUF DMA.
    t1s = prep.tile([P, J], I32)
    for pb in range(4):
        nc.sync.dma_start(t1s[pb * K:pb * K + 16, :],
                          t1[pb * K + 16:(pb + 1) * K, :])
    # W16[r, col] = stream[col*16+r]; col = 2m+khi, m = a*128 + pb*32 + j
    w16s = prep.tile([P, J * 8], I16)
    w16sv = w16s[:].rearrange("p (a pj two) -> p a pj two", a=16, pj=P, two=2)
    t1v = t1[:].rearrange("p (a j) -> p a j", j=K)
    t1sv = t1s[:].rearrange("p (a j) -> p a j", j=K)
    for pb in range(4):
        nc.vector.tensor_copy(w16sv[0:16, :, pb * K:(pb + 1) * K, 0],
                              t1v[pb * K:pb * K + 16, :, :])
        nc.vector.tensor_copy(w16sv[0:16, :, pb * K:(pb + 1) * K, 1],
                              t1sv[pb * K:pb * K + 16, :, :])
    # replicate the 16-row wrap into all 8 gpsimd core groups via DRAM bounce
    nc.sync.dma_start(w16_d[:], w16s[0:16, :])
    w16 = consts.tile([P, J * 8], I16)  # [128, 4096]
    nc.sync.dma_start(w16[:], w16_d[:].partition_broadcast(8))

    # ---------------- gather + slotted buffers ----------------
    gbufs = [gpool.tile([P, JS * REC], F32, tag=f"g{i}", name=f"g{i}")
             for i in range(2)]
    sbufs = [gpool.tile([P, JS * 64], BF16, tag=f"s{i}", name=f"s{i}")
             for i in range(2)]
    nc.vector.memset(sbufs[0][:], 0.0)
    nc.vector.memset(sbufs[1][:], 0.0)

    def gather_block(s):
        if STAGE < 2:
            return
        g = gbufs[s % 2]
        nc.gpsimd.dma_gather(
            out_ap=g[:].rearrange("p (j f) -> p j f", f=REC),
            in_ap=rec_d[:],
            idxs_ap=w16[:, s * (JS * 8):(s + 1) * (JS * 8)],
            num_idxs=JS * P,
            num_idxs_reg=JS * P,
            elem_size=REC,
            single_packet=False,
        )

    def subtile(s):
        if STAGE < 2:
            return
        g = gbufs[s % 2]
        sbd = sbufs[s % 2]
        gv = g[:].rearrange("p (j f) -> p j f", f=REC)
        if STAGE < 3:
            if s == 0:
                nc.sync.dma_start(out_d[0:64, :],
                                  gv[0:64, 0, 0:O])
            return

        # --- block-diag data: raw bf16 byte moves on DMA rings (not DVE/ACT)
        gb = g[:].bitcast(BF16).rearrange("p (j f) -> p j f", f=REC * 2)
        sv = sbd[:].rearrange("p (j s c) -> p j s c", s=T, c=C)
        for t in range(T):
            src = gb[t * K:(t + 1) * K, :, 2 * D:2 * D + C]
            dst = sv[t * K:(t + 1) * K, :, t, :]
            eng = nc.sync if t % 2 == 0 else nc.scalar
            eng.dma_start(dst, src)

        # --- delta = q - l
        d3 = dve.tile([P, JS * D], F32, tag="d3")
        q_v = qT3[:].rearrange("p (j d) -> p j d", d=D)
        nc.vector.tensor_sub(
            d3[:].rearrange("p (j d) -> p j d", d=D),
            q_v[:, s * JS:(s + 1) * JS, :],
            gv[:, :, 0:D])

        d3v = d3[:].rearrange("p (j d) -> p j d", d=D)
        dx, dy, dz = d3v[:, :, 0], d3v[:, :, 1], d3v[:, :, 2]

        # --- s2 = dx^2+dy^2+dz^2
        s2 = dve.tile([P, JS], F32, tag="s2")
        tmp = dve.tile([P, JS], F32, tag="tmp")
        nc.vector.tensor_mul(s2[:], dx, dx)
        nc.vector.tensor_mul(tmp[:], dy, dy)
        nc.vector.tensor_add(s2[:], s2[:], tmp[:])
        nc.vector.tensor_mul(tmp[:], dz, dz)
        nc.vector.tensor_add(s2[:], s2[:], tmp[:])

        # --- per-axis terms p*[j,e] = 2*off*d + off^2 (+ s2 on x)
        def axis_term(dcomp, add_s2, tg):
            pt = dve.tile([P, JS * D], F32, tag=tg)
            ptv = pt[:].rearrange("p (j e) -> p j e", e=D)
            din = dcomp.unsqueeze(2).broadcast_to((P, JS, D))
            oc = oxc[:].unsqueeze(1).broadcast_to((P, JS, D))
            o2 = ox2c[:].unsqueeze(1).broadcast_to((P, JS, D))
            nc.vector.tensor_mul(ptv, din, oc)
            nc.vector.tensor_add(ptv, ptv, o2)
            if add_s2:
                s2b = s2[:].unsqueeze(2).broadcast_to((P, JS, D))
                nc.vector.tensor_add(ptv, ptv, s2b)
            return pt

        pxe = axis_term(dx, True, "pxe")
        pye = axis_term(dy, False, "pye")
        pze = axis_term(dz, False, "pze")

        # --- u2[j,ex,ey] = pxe+pye ; d2[j,ex,ey,ez] = u2+pze
        u2 = dve.tile([P, JS * 9], F32, tag="u2")
        u2v = u2[:].rearrange("p (j a b) -> p j a b", a=D, b=D)
        nc.vector.tensor_add(
            u2v,
            pxe[:].rearrange("p (j a) -> p j a", a=D).unsqueeze(3)
                  .broadcast_to((P, JS, D, D)),
            pye[:].rearrange("p (j b) -> p j b", b=D).unsqueeze(2)
                  .broadcast_to((P, JS, D, D)))
        d2 = kvp.tile([P, JS * E], F32, tag="d2")
        d2v = d2[:].rearrange("p (j a b) -> p j a b", a=9, b=D)
        nc.vector.tensor_add(
            d2v,
            u2[:].rearrange("p (j a) -> p j a", a=9).unsqueeze(3)
                 .broadcast_to((P, JS, 9, D)),
            pze[:].rearrange("p (j b) -> p j b", b=D).unsqueeze(2)
                  .broadcast_to((P, JS, 9, D)))

        # --- kv = relu(1 - sqrt(d2+eps)/R)^3
        nc.scalar.activation(d2[:], d2[:], AF.Sqrt, bias=epsb[:])
        nc.scalar.activation(d2[:], d2[:], AF.Relu, bias=oneb[:],
                             scale=-1.0 / RADIUS)
        sq = kvp.tile([P, JS * E], F32, tag="sq")
        nc.vector.tensor_mul(sq[:], d2[:], d2[:])
        kv = kvp.tile([P, JS * E], BF16, tag="kvt")
        nc.vector.tensor_mul(kv[:], sq[:], d2[:])

        if STAGE < 4:
            if s == 0:
                nc.sync.dma_start(out_d[0:64, :],
                                  kv[0:64, 0:O])
            return

        # --- acc[(t,c), e] per chunk on PE (bf16: single-pass PE matmul)
        kvv = kv[:].rearrange("p (j e) -> p j e", e=E)
        acc_sb = acc4[s % 4]
        for grp in range(JS // 16):
            ap_ps = accps.tile([64, 16 * E], F32, tag="accps")
            for jl in range(16):
                jj = grp * 16 + jl
                nc.tensor.matmul(ap_ps[:, jl * E:(jl + 1) * E],
                                 sbd[:, jj * 64:(jj + 1) * 64],
                                 kvv[:, jj, :],
                                 start=True, stop=True)
            nc.scalar.activation(acc_sb[:, grp * 16 * E:(grp + 1) * 16 * E],
                                 ap_ps[:], AF.Copy)

        if STAGE < 5:
            if s == 0:
                nc.sync.dma_start(out_d[0:64, :], acc_sb[:, 0:O])
            return

    def final_group(fg):
        # --- out[(t,o), jtot] = sum_e Wbd_e @ acc_e over 4 subtiles (256 cols)
        JT = 4 * JS
        op = outps.tile([64, JT], F32, tag="outps")
        accv = accbig[:].rearrange("p (jt e) -> p jt e", e=E)
        for e in range(E):
            nc.tensor.matmul(op[:], wbd[:, e * 64:(e + 1) * 64],
                             accv[:, :, e],
                             start=(e == 0), stop=(e == E - 1))
        osb = outs.tile([64, JT], F32, tag="osb")
        nc.scalar.activation(osb[:], op[:], AF.Identity, bias=bias4[:])

        # --- transpose to [j, (t,o)] and store contiguously
        out_v = out_d[:].rearrange("(s j t) o -> s j (t o)", s=NSUB, t=T)
        for q in range(4):
            trp = trps.tile([64, 64], F32, tag="trp")
            nc.tensor.transpose(trp[:], osb[:, q * 64:(q + 1) * 64], ident[:])
            trs = outs.tile([64, 64], F32, tag="trs")
            nc.scalar.activation(trs[:], trp[:], AF.Copy)
            nc.sync.dma_start(out_v[fg * 4 + q], trs[:])

    if STAGE < 2:
        nc.sync.dma_start(out_d[0:128, 0:8],
                          w16[:, 0:8].bitcast(F32).rearrange("p (a b) -> p a b", b=1)[:, :, 0]
                          if False else qT3[:, 0:8])
    accbig = accs.tile([64, 4 * JS * E], BF16, tag="accbig", name="accbig")
    acc4 = [accbig[:, i * JS * E:(i + 1) * JS * E] for i in range(4)]

    # ---------------- pipeline ----------------
    gather_block(0)
    for s in range(NSUB):
        if s + 1 < NSUB:
            gather_block(s + 1)
        subtile(s)
        if STAGE >= 5 and s % 4 == 3:
            final_group(s // 4)


_PROGRAM = None


def _get_program():
    global _PROGRAM
    if _PROGRAM is None:
        _PROGRAM = build_program()
    return _PROGRAM


def kernel(qlocs, locs, data, neighbors, weight, bias):
    B, M = qlocs.shape[0], qlocs.shape[1]
    assert (B, M) == (2, 8192)
    ncores = 8

    # host-side constant/layout prep: block-diagonal weights + replicated bias
    wbd = np.zeros((E, 64, 64), np.float32)
    w = np.asarray(weight, np.float32)           # [O, C, E]
    for t in range(T):
        # wbd[e, (t,c), (t,o)] = w[o, c, e]
        wbd[:, t * C:(t + 1) * C, t * O:(t + 1) * O] = w.transpose(2, 1, 0)
    import ml_dtypes
    wbd = np.ascontiguousarray(
        wbd.transpose(1, 0, 2).reshape(64, E * 64)).astype(ml_dtypes.bfloat16)
    bias4 = np.tile(np.asarray(bias, np.float32), T)

    in_maps = []
    for cid in range(ncores):
        b, qq = cid // 4, cid % 4
        sl = slice(qq * NQ, (qq + 1) * NQ)
        in_maps.append({
            "qlocs": np.ascontiguousarray(qlocs[b, sl], np.float32),
            "nbrs": np.ascontiguousarray(neighbors[b, sl], np.int32),
            "locs": np.ascontiguousarray(locs[b], np.float32),
            "data": np.ascontiguousarray(data[b], np.float32),
            "wbd": wbd,
            "bias4": bias4,
        })

    nc = _get_program()
    res = run_bass_kernel_spmd(nc, in_maps, list(range(ncores)),
                               trace=bool(int(os.environ.get("CONVSP_TRACE", "0"))))
    out = np.zeros((B, M, O), np.float32)
    for cid in range(ncores):
        b, qq = cid // 4, cid % 4
        out[b, qq * NQ:(qq + 1) * NQ] = res.results[cid]["out"]
    kernel.last_results = res
    return out



# revision 21
# speedup vs baseline: 2.4870x; 2.4757x over previous
"""ConvSP (SPH message-passing conv) Trainium2 kernel.

Problem (per full input): B=2 batches, N=8192 particles, M=8192 queries,
K=32 neighbors, C=16 in channels, O=16 out channels, 27 kernel cells.

    out[b,m,o] = bias[o] + sum_{e,k,c} kv(b,m,e,k) * data[b, nbr[b,m,k], c] * W[o,c,e]
    kv = relu(1 - sqrt(|qloc + off_e - loc_nbr|^2 + 1e-12)/R)^3

Sharding: 8 cores = 2 batches x 4 query-quarters (2048 queries each), SPMD.

Per-core dataflow (chunk = 4 queries m=4j+t; partition (t,k) = t*32+k):
  - records rec64[n] = [lx,ly,lz, data0..15, pad..] (256B rows) built by
    DRAM->DRAM DMAs; neighbor records fetched with dma_gather: the natural
    flat neighbor stream s = m*32+k lands at out[(t,k), j, :]
    (p = s%128, chunk = s//128).
  - distances on DVE via the separable cell-offset factorization
    d2 = |delta|^2 + sum_axis(2*off*delta + off^2); kv = relu(1-d/R)^3
    via ACT sqrt + ACT relu-affine + DVE squares.
  - data c-fields copied into a block-diagonal "slotted" tile (zeros
    elsewhere, memset once); acc[(t,c), e] per chunk via one PE matmul
    (lhsT = slotted data [128, 64], rhs = dense kv [128, 27]).
  - out[(t,o), j] via 27 accumulated PE matmuls against host-prepped
    block-diagonal W; bias fused in the ACT PSUM->SBUF copy; PE transpose
    to [j, (t,o)] for a contiguous store.
"""
import os
import sys
import numpy as np
from contextlib import ExitStack

sys.path.insert(0, "/opt/trn_rl_repo")

import concourse.bass as bass
import concourse.bacc as bacc
import concourse.mybir as mybir
import concourse.tile as tile
from concourse.masks import make_identity
from concourse.bass_utils import run_bass_kernel_spmd

F32 = mybir.dt.float32
F32R = mybir.dt.float32r
BF16 = mybir.dt.bfloat16
I32 = mybir.dt.int32
I16 = mybir.dt.int16
AF = mybir.ActivationFunctionType

P = 128          # partitions
NQ = 2048        # queries per core
N = 8192         # particles per batch
K = 32           # neighbors
C = 16           # in channels
O = 16           # out channels
D = 3
E = 27           # cells
REC = 64         # padded record fields (f32): lx,ly,lz,d0..15,pad -> 256B
T = 4            # queries per chunk
J = NQ // T      # chunks per core = 512
JS = 64          # chunks per gather block == compute subtile
NSUB = J // JS   # 8
RADIUS = 0.1
DIL = 0.05
STAGE = int(os.environ.get("CONVSP_STAGE", "5"))


def build_program():
    nc = bacc.Bacc("TRN2", target_bir_lowering=False, debug=False,
                   num_devices=8, num_swdge_queues=4)

    qlocs_d = nc.declare_dram_parameter("qlocs", [NQ, D], F32, isOutput=False)
    nbrs_d = nc.declare_dram_parameter("nbrs", [NQ, K], I32, isOutput=False)
    locs_d = nc.declare_dram_parameter("locs", [N, D], F32, isOutput=False)
    data_d = nc.declare_dram_parameter("data", [N, C], F32, isOutput=False)
    wbd_d = nc.declare_dram_parameter("wbd", [64, E * 64], BF16, isOutput=False)
    bias4_d = nc.declare_dram_parameter("bias4", [64], F32, isOutput=False)
    out_d = nc.declare_dram_parameter("out", [NQ, O], F32, isOutput=True)

    rec_d = nc.dram_tensor("rec_scratch", [N, REC], F32)
    q4_d = nc.dram_tensor("q4_scratch", [T, J, D], F32)
    w16_d = nc.dram_tensor("w16_scratch", [16, J * 8], I16)

    with tile.TileContext(nc) as tc:
        with ExitStack() as ctx:
            _build(ctx, tc, qlocs_d, nbrs_d, locs_d, data_d, wbd_d, bias4_d,
                   out_d, rec_d, q4_d, w16_d)
    nc.finalize()
    return nc


def _build(ctx, tc, qlocs_d, nbrs_d, locs_d, data_d, wbd_d, bias4_d,
           out_d, rec_d, q4_d, w16_d):
    nc = tc.nc

    consts = ctx.enter_context(tc.tile_pool(name="consts", bufs=1))
    prep = ctx.enter_context(tc.tile_pool(name="prep", bufs=1))
    gpool = ctx.enter_context(tc.tile_pool(name="gpool", bufs=1))
    dve = ctx.enter_context(tc.tile_pool(name="dve", bufs=2))
    kvp = ctx.enter_context(tc.tile_pool(name="kv", bufs=2))
    accs = ctx.enter_context(tc.tile_pool(name="accs", bufs=1))
    outs = ctx.enter_context(tc.tile_pool(name="outs", bufs=2))
    accps = ctx.enter_context(tc.tile_pool(name="accps", bufs=4, space="PSUM"))
    outps = ctx.enter_context(tc.tile_pool(name="outps", bufs=2, space="PSUM"))
    trps = ctx.enter_context(tc.tile_pool(name="trps", bufs=2, space="PSUM"))

    # ---------------- constants ----------------
    oxc = consts.tile([P, D], F32)      # 2*off(e)
    ox2c = consts.tile([P, D], F32)     # off(e)^2
    for i in range(D):
        off = (i - 1) * DIL
        nc.vector.memset(oxc[:, i:i + 1], 2.0 * off)
        nc.vector.memset(ox2c[:, i:i + 1], off * off)
    epsb = consts.tile([P, 1], F32)
    nc.vector.memset(epsb[:], 1e-12)
    oneb = consts.tile([P, 1], F32)
    nc.vector.memset(oneb[:], 1.0)
    ident = consts.tile([64, 64], F32)
    make_identity(nc, ident[:])
    bias4 = consts.tile([64, 1], F32)
    nc.sync.dma_start(bias4[:], bias4_d[:].rearrange("(p o) -> p o", o=1))
    wbd = consts.tile([64, E * 64], BF16)
    nc.scalar.dma_start(wbd[:], wbd_d[:])

    # ---------------- stage A ----------------
    # rec64[n] = [lx,ly,lz, d0..15, garbage...] built in SBUF, one big write
    tl_locs = prep.tile([P, (N // P) * D], F32)
    nc.scalar.dma_start(tl_locs[:], locs_d[:].rearrange("(p a) d -> p (a d)", p=P))
    tl_data = prep.tile([P, (N // P) * C], F32)
    nc.scalar.dma_start(tl_data[:], data_d[:].rearrange("(p a) c -> p (a c)", p=P))
    tl_rec = prep.tile([P, (N // P) * REC], F32)
    rec_v = tl_rec[:].rearrange("p (a f) -> p a f", f=REC)
    locs_v = tl_locs[:].rearrange("p (a d) -> p a d", d=D)
    data_v = tl_data[:].rearrange("p (a c) -> p a c", c=C)
    for d in range(D):
        nc.vector.tensor_copy(rec_v[:, :, d], locs_v[:, :, d])
    # data fields stored bf16 at byte offset 12 so slot copies are raw
    # byte moves (DMA-able); trailing record bytes stay garbage (never read)
    rec_b = tl_rec[:].bitcast(BF16).rearrange("p (a f) -> p a f", f=REC * 2)
    nc.vector.tensor_copy(rec_b[:, :, 2 * D:2 * D + C], data_v[:, :, :])
    rec_st = rec_d[:].rearrange("(p a) f -> p (a f)", p=P)
    half = (N // P) * REC // 2
    nc.scalar.dma_start(rec_st[:, 0:half], tl_rec[:, 0:half])
    nc.sync.dma_start(rec_st[:, half:], tl_rec[:, half:])

    # q4_d[t, j, d] = qlocs[4j+t, d] via DRAM->DRAM, then broadcast to qT3
    nc.sync.dma_start(q4_d[:], qlocs_d[:].rearrange("(j t) d -> t j d", t=T))
    qT3 = consts.tile([P, J * D], F32)
    for t in range(T):
        src = q4_d[t].rearrange("j d -> (j d)")
        eng = nc.sync if t % 2 == 0 else nc.scalar
        eng.dma_start(qT3[t * K:(t + 1) * K, :], src.partition_broadcast(K))

    # wrap-format gather indices W16[r, 2m+khi] = nbrs[m, khi*16+r], int16,
    # replicated into all 8 gpsimd core groups.
    n1 = prep.tile([P, J], I32)
    nc.sync.dma_start(n1[:].rearrange("p (a k) -> p a k", k=K),
                      nbrs_d[:].rearrange("(a p) k -> p a k", p=P))
    t1 = prep.tile([P, J], I32)
    nc.vector.transpose(t1[:], n1[:])   # t1[(pb,k),(a,j)] = nbrs[a*128+pb*32+j, k]
    # engine APs need 32-aligned partition bases: shift the khi=1 half-rows
    # (k=16..32 of each pb block) down to base pb*32 via SBUF->SBUF DMA.
    t1s = prep.tile([P, J], I32)
    for pb in range(4):
        nc.sync.dma_start(t1s[pb * K:pb * K + 16, :],
                          t1[pb * K + 16:(pb + 1) * K, :])
    # W16[r, col] = stream[col*16+r]; col = 2m+khi, m = a*128 + pb*32 + j
    w16s = prep.tile([P, J * 8], I16)
    w16sv = w16s[:].rearrange("p (a pj two) -> p a pj two", a=16, pj=P, two=2)
    t1v = t1[:].rearrange("p (a j) -> p a j", j=K)
    t1sv = t1s[:].rearrange("p (a j) -> p a j", j=K)
    for pb in range(4):
        nc.vector.tensor_copy(w16sv[0:16, :, pb * K:(pb + 1) * K, 0],
                              t1v[pb * K:pb * K + 16, :, :])
        nc.vector.tensor_copy(w16sv[0:16, :, pb * K:(pb + 1) * K, 1],
                              t1sv[pb * K:pb * K + 16, :, :])
    # replicate the 16-row wrap into all 8 gpsimd core groups via DRAM bounce
    nc.sync.dma_start(w16_d[:], w16s[0:16, :])
    w16 = consts.tile([P, J * 8], I16)  # [128, 4096]
    nc.sync.dma_start(w16[:], w16_d[:].partition_broadcast(8))

    # ---------------- gather + slotted buffers ----------------
    gbufs = [gpool.tile([P, JS * REC], F32, tag=f"g{i}", name=f"g{i}")
             for i in range(2)]
    sbufs = [gpool.tile([P, JS * 64], BF16, tag=f"s{i}", name=f"s{i}")
             for i in range(2)]
    nc.vector.memset(sbufs[0][:], 0.0)
    nc.vector.memset(sbufs[1][:], 0.0)

    def gather_block(s):
        if STAGE < 2:
            return
        g = gbufs[s % 2]
        gvr = g[:].rearrange("p (j f) -> p j f", f=REC)
        NSPL = 4           # spread the block's records over 4 SWDGE queues
        cpq = JS // NSPL
        icols = JS * 8 // NSPL
        for q in range(NSPL):
            nc.gpsimd.dma_gather(
                out_ap=gvr[:, q * cpq:(q + 1) * cpq, :],
                in_ap=rec_d[:],
                idxs_ap=w16[:, s * (JS * 8) + q * icols:
                            s * (JS * 8) + (q + 1) * icols],
                num_idxs=cpq * P,
                num_idxs_reg=cpq * P,
                elem_size=REC,
                single_packet=False,
                queue_num=q,
            )

    def subtile(s):
        if STAGE < 2:
            return
        g = gbufs[s % 2]
        sbd = sbufs[s % 2]
        gv = g[:].rearrange("p (j f) -> p j f", f=REC)
        if STAGE < 3:
            if s == 0:
                nc.sync.dma_start(out_d[0:64, :],
                                  gv[0:64, 0, 0:O])
            return

        # --- block-diag data: raw bf16 byte moves on DMA rings (not DVE/ACT)
        gb = g[:].bitcast(BF16).rearrange("p (j f) -> p j f", f=REC * 2)
        sv = sbd[:].rearrange("p (j s c) -> p j s c", s=T, c=C)
        for t in range(T):
            src = gb[t * K:(t + 1) * K, :, 2 * D:2 * D + C]
            dst = sv[t * K:(t + 1) * K, :, t, :]
            eng = nc.sync if t % 2 == 0 else nc.scalar
            eng.dma_start(dst, src)

        # --- delta = q - l
        d3 = dve.tile([P, JS * D], F32, tag="d3")
        q_v = qT3[:].rearrange("p (j d) -> p j d", d=D)
        nc.vector.tensor_sub(
            d3[:].rearrange("p (j d) -> p j d", d=D),
            q_v[:, s * JS:(s + 1) * JS, :],
            gv[:, :, 0:D])

        d3v = d3[:].rearrange("p (j d) -> p j d", d=D)
        dx, dy, dz = d3v[:, :, 0], d3v[:, :, 1], d3v[:, :, 2]

        # --- s2 = dx^2+dy^2+dz^2
        s2 = dve.tile([P, JS], F32, tag="s2")
        tmp = dve.tile([P, JS], F32, tag="tmp")
        nc.vector.tensor_mul(s2[:], dx, dx)
        nc.vector.tensor_mul(tmp[:], dy, dy)
        nc.vector.tensor_add(s2[:], s2[:], tmp[:])
        nc.vector.tensor_mul(tmp[:], dz, dz)
        nc.vector.tensor_add(s2[:], s2[:], tmp[:])

        # --- per-axis terms p*[j,e] = 2*off*d + off^2 (+ s2 on x)
        def axis_term(dcomp, add_s2, tg):
            pt = dve.tile([P, JS * D], F32, tag=tg)
            ptv = pt[:].rearrange("p (j e) -> p j e", e=D)
            din = dcomp.unsqueeze(2).broadcast_to((P, JS, D))
            oc = oxc[:].unsqueeze(1).broadcast_to((P, JS, D))
            o2 = ox2c[:].unsqueeze(1).broadcast_to((P, JS, D))
            nc.vector.tensor_mul(ptv, din, oc)
            nc.vector.tensor_add(ptv, ptv, o2)
            if add_s2:
                s2b = s2[:].unsqueeze(2).broadcast_to((P, JS, D))
                nc.vector.tensor_add(ptv, ptv, s2b)
            return pt

        pxe = axis_term(dx, True, "pxe")
        pye = axis_term(dy, False, "pye")
        pze = axis_term(dz, False, "pze")

        # --- u2[j,ex,ey] = pxe+pye ; d2[j,ex,ey,ez] = u2+pze
        u2 = dve.tile([P, JS * 9], F32, tag="u2")
        u2v = u2[:].rearrange("p (j a b) -> p j a b", a=D, b=D)
        nc.vector.tensor_add(
            u2v,
            pxe[:].rearrange("p (j a) -> p j a", a=D).unsqueeze(3)
                  .broadcast_to((P, JS, D, D)),
            pye[:].rearrange("p (j b) -> p j b", b=D).unsqueeze(2)
                  .broadcast_to((P, JS, D, D)))
        d2 = kvp.tile([P, JS * E], F32, tag="d2")
        d2v = d2[:].rearrange("p (j a b) -> p j a b", a=9, b=D)
        nc.vector.tensor_add(
            d2v,
            u2[:].rearrange("p (j a) -> p j a", a=9).unsqueeze(3)
                 .broadcast_to((P, JS, 9, D)),
            pze[:].rearrange("p (j b) -> p j b", b=D).unsqueeze(2)
                  .broadcast_to((P, JS, 9, D)))

        # --- kv = relu(1 - sqrt(d2+eps)/R)^3
        nc.scalar.activation(d2[:], d2[:], AF.Sqrt, bias=epsb[:])
        nc.scalar.activation(d2[:], d2[:], AF.Relu, bias=oneb[:],
                             scale=-1.0 / RADIUS)
        sq = kvp.tile([P, JS * E], F32, tag="sq")
        nc.vector.tensor_mul(sq[:], d2[:], d2[:])
        kv = kvp.tile([P, JS * E], BF16, tag="kvt")
        nc.vector.tensor_mul(kv[:], sq[:], d2[:])

        if STAGE < 4:
            if s == 0:
                nc.sync.dma_start(out_d[0:64, :],
                                  kv[0:64, 0:O])
            return

        # --- acc[(t,c), e] per chunk on PE (bf16: single-pass PE matmul)
        kvv = kv[:].rearrange("p (j e) -> p j e", e=E)
        acc_sb = acc4[s % 4]
        for grp in range(JS // 16):
            ap_ps = accps.tile([64, 16 * E], F32, tag="accps")
            for jl in range(16):
                jj = grp * 16 + jl
                nc.tensor.matmul(ap_ps[:, jl * E:(jl + 1) * E],
                                 sbd[:, jj * 64:(jj + 1) * 64],
                                 kvv[:, jj, :],
                                 start=True, stop=True)
            nc.scalar.activation(acc_sb[:, grp * 16 * E:(grp + 1) * 16 * E],
                                 ap_ps[:], AF.Copy)

        if STAGE < 5:
            if s == 0:
                nc.sync.dma_start(out_d[0:64, :], acc_sb[:, 0:O])
            return

    def final_group(fg):
        # --- out[(t,o), jtot] = sum_e Wbd_e @ acc_e over 4 subtiles (256 cols)
        JT = 4 * JS
        op = outps.tile([64, JT], F32, tag="outps")
        accv = accbig[:].rearrange("p (jt e) -> p jt e", e=E)
        for e in range(E):
            nc.tensor.matmul(op[:], wbd[:, e * 64:(e + 1) * 64],
                             accv[:, :, e],
                             start=(e == 0), stop=(e == E - 1))
        osb = outs.tile([64, JT], F32, tag="osb")
        nc.scalar.activation(osb[:], op[:], AF.Identity, bias=bias4[:])

        # --- transpose to [j, (t,o)] and store contiguously
        out_v = out_d[:].rearrange("(s j t) o -> s j (t o)", s=NSUB, t=T)
        for q in range(4):
            trp = trps.tile([64, 64], F32, tag="trp")
            nc.tensor.transpose(trp[:], osb[:, q * 64:(q + 1) * 64], ident[:])
            trs = outs.tile([64, 64], F32, tag="trs")
            nc.scalar.activation(trs[:], trp[:], AF.Copy)
            nc.sync.dma_start(out_v[fg * 4 + q], trs[:])

    if STAGE < 2:
        nc.sync.dma_start(out_d[0:128, 0:8],
                          w16[:, 0:8].bitcast(F32).rearrange("p (a b) -> p a b", b=1)[:, :, 0]
                          if False else qT3[:, 0:8])
    accbig = accs.tile([64, 4 * JS * E], BF16, tag="accbig", name="accbig")
    acc4 = [accbig[:, i * JS * E:(i + 1) * JS * E] for i in range(4)]

    # ---------------- pipeline ----------------
    gather_block(0)
    for s in range(NSUB):
        if s + 1 < NSUB:
            gather_block(s + 1)
        subtile(s)
        if STAGE >= 5 and s % 4 == 3:
            final_group(s // 4)


_PROGRAM = None


def _get_program():
    global _PROGRAM
    if _PROGRAM is None:
        _PROGRAM = build_program()
    return _PROGRAM


def kernel(qlocs, locs, data, neighbors, weight, bias):
    B, M = qlocs.shape[0], qlocs.shape[1]
    assert (B, M) == (2, 8192)
    ncores = 8

    # host-side constant/layout prep: block-diagonal weights + replicated bias
    wbd = np.zeros((E, 64, 64), np.float32)
    w = np.asarray(weight, np.float32)           # [O, C, E]
    for t in range(T):
        # wbd[e, (t,c), (t,o)] = w[o, c, e]
        wbd[:, t * C:(t + 1) * C, t * O:(t + 1) * O] = w.transpose(2, 1, 0)
    import ml_dtypes
    wbd = np.ascontiguousarray(
        wbd.transpose(1, 0, 2).reshape(64, E * 64)).astype(ml_dtypes.bfloat16)
    bias4 = np.tile(np.asarray(bias, np.float32), T)

    in_maps = []
    for cid in range(ncores):
        b, qq = cid // 4, cid % 4
        sl = slice(qq * NQ, (qq + 1) * NQ)
        in_maps.append({
            "qlocs": np.ascontiguousarray(qlocs[b, sl], np.float32),
            "nbrs": np.ascontiguousarray(neighbors[b, sl], np.int32),
            "locs": np.ascontiguousarray(locs[b], np.float32),
            "data": np.ascontiguousarray(data[b], np.float32),
            "wbd": wbd,
            "bias4": bias4,
        })

    nc = _get_program()
    res = run_bass_kernel_spmd(nc, in_maps, list(range(ncores)),
                               trace=bool(int(os.environ.get("CONVSP_TRACE", "0"))))
    out = np.zeros((B, M, O), np.float32)
    for cid in range(ncores):
        b, qq = cid // 4, cid % 4
        out[b, qq * NQ:(qq + 1) * NQ] = res.results[cid]["out"]
    kernel.last_results = res
    return out

